# revision 35
# baseline (speedup 1.0000x reference)
"""Trainium2 Bass kernel for nn_DeepModel_multi_12945031430869.

Computes, for heads h in 0..31:
    y[:, h] = relu(x @ W1[h] + b1[h]) @ W2[h] + b2[h]
    out[:, h*513:(h+1)*513] = [x, y[:, h]]          # [4096, 16416]

Sharding: head-parallel across 8 NeuronCores (4 heads per core). Each core
computes only its y columns [4096, 4]; the host assembles the full output
(the x column blocks are pure replication, done in numpy) and adds b2.

Per-core device program, balanced around the PE roofline:
  - Contraction (d=512) split by precision: d in [0,256) as 2 bf16
    matmuls, d in [256,512) as ONE fp8-e4m3 DoubleRow matmul (2 moving
    rows/cycle = 2x bf16 rate, verified on HW).  Half-fp8 keeps the
    deterministic max-rel error at 1.7e-2 (< 2e-2 gate; full fp8 would
    be 2.4e-2, full bf16 1.7e-3 but 28% slower).
  - Activation engine pre-loads each PSUM tile with the bias b1 (bf16
    broadcast Copy), so all matmuls accumulate on top (start=False).
    PSUM banks are warmed with a dummy start=True group first: a fresh
    bank's first matmul write discards preloaded values otherwise.
  - DVE: one fused scalar_tensor_tensor per [128,1024] PSUM tile:
        sc = max(ps, 0) * w2bc ; part[:, j] = sum_f(sc)
    and a tiny tensor_reduce per (head, row-tile) producing y.
  - Engine busy (per core): PE ~348us, DVE ~326us, Act ~289us.
"""

import numpy as np

N = 4096
D_IN = 512
D_H = 2048
USED = 32
NCORES = 8
HPC = USED // NCORES  # heads per core = 4
RT = N // 128         # row tiles = 32

_PROG = None


def _install_trace_hook():
    """Register the axon NTFF profiling hook so run_bass_kernel_spmd can
    report HW exec time.  The agent image's antenv package lacks
    axon_hooks; synthesize it and point it at libaxon_pjrt.so.  Safe
    no-op outside that environment."""
    import sys
    import types

    try:
        import antenv
    except ImportError:
        return
    if not hasattr(antenv, "axon_hooks"):
        mod = types.ModuleType("antenv.axon_hooks")
        mod._HOOK = None
        mod.set_axon_ntff_profile_hook = lambda hook: setattr(mod, "_HOOK", hook)
        mod.get_axon_ntff_profile_hook = lambda: mod._HOOK
        sys.modules["antenv.axon_hooks"] = mod
        antenv.axon_hooks = mod
    import antenv.axon_hooks as ah

    if ah.get_axon_ntff_profile_hook() is None:
        from trn_agent_boot.trn_boot import _ntff_profile_via_ctypes

        ah.set_axon_ntff_profile_hook(
            _ntff_profile_via_ctypes("/opt/axon/libaxon_pjrt.so")
        )


def _build_program():
    import concourse.tile as tile
    import concourse.mybir as mybir
    from concourse import bacc

    f32 = mybir.dt.float32
    bf16 = mybir.dt.bfloat16
    fp8 = mybir.dt.float8e4
    DR = mybir.MatmulPerfMode.DoubleRow

    nc = bacc.Bacc("TRN2", target_bir_lowering=False, debug=False)

    # Contraction split: d in [0,256) as 2 bf16 matmuls, d in [256,512) as
    # one fp8-e4m3 DoubleRow matmul (2 rows/cycle on the PE = 2x bf16).
    xT_d = nc.dram_tensor("xT", [2, 128, N], bf16, kind="ExternalInput").ap()
    x8_d = nc.dram_tensor("x8", [128, 2, N], fp8, kind="ExternalInput").ap()
    w1_d = nc.dram_tensor("w1", [HPC, 2, 128, D_H], bf16, kind="ExternalInput").ap()
    w18_d = nc.dram_tensor("w18", [HPC, 128, 2, D_H], fp8, kind="ExternalInput").ap()
    b1_d = nc.dram_tensor("b1bc", [HPC, 128, D_H], bf16, kind="ExternalInput").ap()
    w2_d = nc.dram_tensor("w2bc", [HPC, 128, D_H], bf16, kind="ExternalInput").ap()
    # y laid out head-major [h][p][rt] to match y_all's flat SBUF order, so
    # each head's result ships as one contiguous 16KB DMA; host transposes.
    out_d = nc.dram_tensor("out", [HPC, 128, RT], f32, kind="ExternalOutput").ap()

    mx = mybir.AluOpType.max
    mult = mybir.AluOpType.mult
    add = mybir.AluOpType.add
    copy_f = mybir.ActivationFunctionType.Copy
    ax_x = mybir.AxisListType.X

    with tile.TileContext(nc) as tc:
        with tc.tile_pool(name="xt", bufs=1) as xtp, \
             tc.tile_pool(name="cst", bufs=1) as cst, \
             tc.tile_pool(name="w1p", bufs=4) as w1p, \
             tc.tile_pool(name="w18p", bufs=2) as w18p, \
             tc.tile_pool(name="b1p", bufs=3) as b1p, \
             tc.tile_pool(name="w2p", bufs=3) as w2p, \
             tc.tile_pool(name="ps", bufs=4, space="PSUM") as pp, \
             tc.tile_pool(name="scr", bufs=4) as scr, \
             tc.tile_pool(name="prt", bufs=4) as prt:

            # Per-head streamed tiles.  Lead-in order matters: b1[0] first
            # (the Act pre-init needs it before anything else, and it is
            # small), then xT / W1[0] interleaved so the first tiles can
            # start while the rest of xT is still in flight.
            w1t = {}
            b1t = {}
            w2t = {}

            # Head-0 lead-in, split into halves ordered so the first
            # (rt=0, j=0) tile group's operands land first.
            b = b1p.tile([128, D_H], bf16, tag="b1")
            nc.sync.dma_start(b[:, 0:1024], b1_d[0, :, 0:1024])
            b1t[0] = b
            xts = []
            w1t[0] = []
            w18t = {}
            for k in range(2):
                t = xtp.tile([128, N], bf16, tag=f"x{k}")
                nc.sync.dma_start(t[:, 0:2048], xT_d[k, :, 0:2048])
                xts.append(t)
                w = w1p.tile([128, D_H], bf16, tag="w1")
                nc.sync.dma_start(w[:, 0:1024], w1_d[0, k, :, 0:1024])
                w1t[0].append(w)
            x8 = xtp.tile([128, 2, N], fp8, tag="x8")
            nc.sync.dma_start(x8[:, :, 0:2048], x8_d[:, :, 0:2048])
            w8 = w18p.tile([128, 2, D_H], fp8, tag="w18")
            nc.sync.dma_start(w8[:, :, 0:1024], w18_d[0, :, :, 0:1024])
            w18t[0] = w8
            w2 = w2p.tile([128, D_H], bf16, tag="w2")
            nc.sync.dma_start(w2[:, 0:1024], w2_d[0, :, 0:1024])
            w2t[0] = w2
            # second halves
            nc.sync.dma_start(b[:, 1024:2048], b1_d[0, :, 1024:2048])
            for k in range(2):
                nc.sync.dma_start(w1t[0][k][:, 1024:2048], w1_d[0, k, :, 1024:2048])
            nc.sync.dma_start(w8[:, :, 1024:2048], w18_d[0, :, :, 1024:2048])
            nc.sync.dma_start(w2[:, 1024:2048], w2_d[0, :, 1024:2048])
            for k in range(2):
                nc.sync.dma_start(xts[k][:, 2048:4096], xT_d[k, :, 2048:4096])
            nc.sync.dma_start(x8[:, :, 2048:4096], x8_d[:, :, 2048:4096])
            y_all = cst.tile([128, RT * HPC], f32, tag="y")

            # Warm every PSUM buffer with a start=True matmul before its
            # first real use: a fresh bank's first matmul write discards
            # the Act bias pre-load (pending-zero), so first-touch must be
            # a complete start/stop group.  Runs during the DMA lead-in.
            zl = cst.tile([1, 128], bf16, tag="zl")
            nc.vector.memset(zl[:], 0.0)
            zr = cst.tile([1, 512], bf16, tag="zr")
            nc.vector.memset(zr[:], 0.0)
            for _ in range(4):
                psw = pp.tile([128, 1024], f32, tag="ps")
                for t in range(2):
                    nc.tensor.matmul(
                        psw[:, t * 512:(t + 1) * 512],
                        lhsT=zl[:], rhs=zr[:], start=True, stop=True,
                    )

            def stage_head(h):
                b = b1p.tile([128, D_H], bf16, tag="b1")
                nc.sync.dma_start(b[:], b1_d[h])
                b1t[h] = b
                ts = []
                for k in range(2):
                    t = w1p.tile([128, D_H], bf16, tag="w1")
                    nc.sync.dma_start(t[:], w1_d[h, k])
                    ts.append(t)
                w1t[h] = ts
                w8_ = w18p.tile([128, 2, D_H], fp8, tag="w18")
                nc.sync.dma_start(w8_[:], w18_d[h])
                w18t[h] = w8_
                w = w2p.tile([128, D_H], bf16, tag="w2")
                nc.sync.dma_start(w[:], w2_d[h])
                w2t[h] = w

            for h in range(HPC):
                if h + 1 < HPC:
                    stage_head(h + 1)
                for rt in range(RT):
                    rs = rt * 128
                    part = prt.tile([128, 2], f32, tag="part")
                    pss = []
                    for j in range(2):
                        ps = pp.tile([128, 1024], f32, tag="ps")
                        nc.scalar.activation(
                            ps[:], b1t[h][:, j * 1024:j * 1024 + 1024], copy_f
                        )
                        pss.append(ps)
                    # k-outer over both j halves: each stationary (lhsT)
                    # load serves four 512-column matmuls before switching.
                    for k in range(2):
                        for j in range(2):
                            for t in range(2):
                                col = t * 512
                                nc.tensor.matmul(
                                    pss[j][:, col:col + 512],
                                    lhsT=xts[k][:, rs:rs + 128],
                                    rhs=w1t[h][k][:, j * 1024 + col:j * 1024 + col + 512],
                                    start=False,
                                    stop=False,
                                    skip_group_check=True,
                                )
                    for j in range(2):
                        for t in range(2):
                            col = t * 512
                            nc.tensor.matmul(
                                pss[j][:, col:col + 512],
                                lhsT=x8[:, :, rs:rs + 128],
                                rhs=w18t[h][:, :, j * 1024 + col:j * 1024 + col + 512],
                                start=False,
                                stop=True,
                                perf_mode=DR,
                                skip_group_check=True,
                            )
                    for j in range(2):
                        sc = scr.tile([128, 1024], f32, tag="sc")
                        nc.vector.scalar_tensor_tensor(
                            out=sc[:],
                            in0=pss[j][:],
                            scalar=0.0,
                            in1=w2t[h][:, j * 1024:j * 1024 + 1024],
                            op0=mx,
                            op1=mult,
                            accum_out=part[:, j:j + 1],
                        )
                    cy = h * RT + rt
                    nc.vector.tensor_reduce(
                        out=y_all[:, cy:cy + 1], in_=part[:], axis=ax_x, op=add
                    )
                    if rt % 8 == 7:
                        q = rt - 7
                        nc.sync.dma_start(
                            out_d[h, :, q:q + 8],
                            y_all[:, h * RT + q:h * RT + q + 8],
                        )

    nc.compile()
    return nc


def _get_program():
    global _PROG
    if _PROG is None:
        _PROG = _build_program()
    return _PROG


def kernel(x, W1, b1, W2, b2):
    import ml_dtypes
    from concourse.bass_utils import run_bass_kernel_spmd

    bf16 = ml_dtypes.bfloat16

    x = np.asarray(x, dtype=np.float32)
    W1 = np.asarray(W1, dtype=np.float32)
    b1 = np.asarray(b1, dtype=np.float32)
    W2 = np.asarray(W2, dtype=np.float32)
    b2 = np.asarray(b2, dtype=np.float32)

    fp8 = ml_dtypes.float8_e4m3fn

    nc = _get_program()

    xT = np.ascontiguousarray(x.T)  # [512, 4096]
    xTr = xT[:256].reshape(2, 128, N).astype(bf16)
    x8r = np.ascontiguousarray(
        xT[256:].reshape(2, 128, N).transpose(1, 0, 2)
    ).astype(fp8)

    in_maps = []
    for c in range(NCORES):
        hs = slice(HPC * c, HPC * (c + 1))
        w1r = np.ascontiguousarray(
            W1[hs][:, :256, :].reshape(HPC, 2, 128, D_H)
        ).astype(bf16)
        w18r = np.ascontiguousarray(
            W1[hs][:, 256:, :].reshape(HPC, 2, 128, D_H).transpose(0, 2, 1, 3)
        ).astype(fp8)
        b1bc = np.ascontiguousarray(
            np.broadcast_to(b1[hs][:, None, :], (HPC, 128, D_H)).astype(bf16)
        )
        w2bc = np.ascontiguousarray(
            np.broadcast_to(W2[hs][:, None, :], (HPC, 128, D_H)).astype(bf16)
        )
        in_maps.append({
            "xT": xTr,
            "x8": x8r,
            "w1": w1r,
            "w18": w18r,
            "b1bc": b1bc,
            "w2bc": w2bc,
        })

    import os
    trace = os.environ.get("BASS_KERNEL_TRACE") == "1"
    try:
        _install_trace_hook()
    except Exception:
        pass
    res = run_bass_kernel_spmd(nc, in_maps, list(range(NCORES)), trace=trace)
    kernel.last_result = res

    y = np.concatenate(
        [
            res.results[c]["out"].transpose(2, 1, 0).reshape(N, HPC)
            for c in range(NCORES)
        ],
        axis=1,
    )  # [N, 32]

    out = np.empty((N, USED * (D_IN + 1)), dtype=np.float32)
    o3 = out.reshape(N, USED, D_IN + 1)
    o3[:, :, :D_IN] = x[:, None, :]
    o3[:, :, D_IN] = y + b2[None, :USED]
    return out
